# revision 14
# baseline (speedup 1.0000x reference)
"""Trainium2 Bass kernel for nn_ARDecoderECD (GRU->LSTM AR decoder).

Strategy (pure data-parallel over 8 NeuronCores, batch-sharded):
  - layout: hidden dim on SBUF partitions, batch on the free dim
  - embedding + GRU input projection folded into a 23-row table applied via
    one-hot matmul (one-hot computed on host in bf16, streamed from DRAM)
  - 2 independent batch chains of 512 per core
  - TWO PASSES: all 26 GRU steps first (hidden states accumulate in a
    persistent SBUF bf16 buffer), then all 26 LSTM steps.  Each pass gets
    4 PSUM banks per chain with no cross-phase bank conflicts, so the two
    chains' serial recurrences overlap cleanly on the engines.
  - LSTM i,f,o gates land contiguously in PSUM -> single fused sigmoid
  - bf16 matmuls and bf16 SBUF elementwise (2x/4x DVE); LSTM c state in f32
  - elementwise spread over ACT / DVE / Pool to balance engine load
"""

import numpy as np
from contextlib import ExitStack

import concourse.bacc as bacc
import concourse.bass as bass
import concourse.tile as tile
from concourse import mybir
from concourse.bass_utils import run_bass_kernel_spmd

B, T = 8192, 26
V, E, H, L = 23, 100, 128, 64
N_CORES = 8
BC = B // N_CORES  # 1024 samples per core
F32 = mybir.dt.float32
F32R = mybir.dt.float32r
BF16 = mybir.dt.bfloat16
AF = mybir.ActivationFunctionType
ALU = mybir.AluOpType
NCH = 2
CW = BC // NCH  # 512


def _emit(nc, tc, d, ctx, reps=1):
    """Emit the per-core kernel. d maps names -> DRAM APs."""
    wp = ctx.enter_context(tc.tile_pool(name="weights", bufs=1))
    run = ctx.enter_context(tc.tile_pool(name="run", bufs=2))
    opool = ctx.enter_context(tc.tile_pool(name="opool", bufs=4))
    pp = ctx.enter_context(tc.tile_pool(name="psum", bufs=1, space="PSUM"))

    def mm(out, lhsT, rhs, start, stop):
        nc.tensor.matmul(out, lhsT, rhs, start=start, stop=stop)

    # ---- load weights ----
    def wload(name, shape, dt_):
        t = wp.tile(shape, dt_, name=name)
        nc.sync.dma_start(t[:], d[name][:])
        return t

    xg_sb = wload("xg_tab", [V, 3 * H], BF16)
    whh_sb = wload("whh_T", [H, 3 * H], BF16)
    fcw_sb = wload("fcw_T", [L, H], F32R)
    fcb_sb = wload("fc_b", [H, 1], F32)
    bhhn_sb = wload("bhh_n", [H, 1], F32)
    wihl_sb = wload("wih_lT", [H, 4 * E], BF16)
    whhl_sb = wload("whh_laug", [E + 1, 4 * E], BF16)
    outw_sb = wload("out_waug", [E + 1, V], BF16)
    zT_sb = wload("zT", [L, BC], F32R)

    chs = [slice(c * CW, (c + 1) * CW) for c in range(NCH)]

    # Persistent 4-bank PSUM tile per chain, manually sliced.
    P = [pp.tile([128, 4 * CW], F32, name=f"P{c}") for c in range(NCH)]
    # GRU hidden states for all T steps (per chain), bf16 in SBUF.
    y = [wp.tile([H, T * CW], BF16, name=f"y{c}") for c in range(NCH)]

    # LSTM state ping-pong tiles (per chain) with persistent ones-row (row E)
    hl_t = [[None, None] for _ in range(NCH)]
    for c in range(NCH):
        for idx in range(2):
            hlx = wp.tile([E + 1, CW], BF16, name=f"hl{c}_{idx}")
            nc.sync.dma_start(hlx[:], d["hl_init"][:, 0:CW])
            hl_t[c][idx] = hlx

    for rep in range(reps):
        h0 = [None] * NCH
        c_prev = [None] * NCH
        O_tiles = {}

        def load_O(t):
            Ot = opool.tile([V, BC], BF16, tag="O", name=f"O{t}_{rep}")
            nc.sync.dma_start(Ot[:], d["O"][t])
            O_tiles[t] = Ot

        def emit_h0(c):
            ph0 = P[c][:, 0:CW]
            mm(ph0, fcw_sb[:], zT_sb[:, chs[c]], start=True, stop=True)
            hc = run.tile([H, CW], BF16, tag=f"h{c}", name=f"h_init_{rep}_{c}")
            nc.scalar.activation(hc[:], ph0, AF.Tanh, bias=fcb_sb[:, 0:1])
            h0[c] = hc

        g_state = [None] * NCH  # (t2, oz, zh) handed from gru_front to gru_back

        def emit_gru_front(t, c):
            Ot = O_tiles[t]
            Pc = P[c]
            h_prev = h0[c][:] if t == 0 else y[c][:, (t - 1) * CW : t * CW]
            prz = Pc[:, 0 : 2 * CW]
            pxn = Pc[:, 2 * CW : 3 * CW]
            phn = Pc[:, 3 * CW : 4 * CW]
            # r's recurrent matmul first so r-sigmoid (the backbone) can
            # start one matmul after h lands; z's parts last (off-path).
            mm(prz[:, 0:CW], xg_sb[:, 0:H], Ot[:, chs[c]],
               start=True, stop=False)
            mm(prz[:, 0:CW], whh_sb[:, 0:H], h_prev, start=False, stop=True)
            mm(pxn, xg_sb[:, 2 * H : 3 * H], Ot[:, chs[c]],
               start=True, stop=True)
            mm(phn, whh_sb[:, 2 * H : 3 * H], h_prev, start=True, stop=True)
            mm(prz[:, CW:], xg_sb[:, H : 2 * H], Ot[:, chs[c]],
               start=True, stop=False)
            mm(prz[:, CW:], whh_sb[:, H : 2 * H], h_prev,
               start=False, stop=True)

            # r-sigmoid alone gates the backbone; z is off-path
            r_sb = run.tile([H, CW], BF16, tag=f"r{c}", name=f"r{t}_{c}_{rep}")
            nc.scalar.activation(r_sb[:], prz[:, 0:CW], AF.Sigmoid)
            z_sb = run.tile([H, CW], BF16, tag=f"z{c}", name=f"z{t}_{c}_{rep}")
            nc.scalar.activation(z_sb[:], prz[:, CW:], AF.Sigmoid)
            # backbone: n-gate pre-activation
            t1_sb = run.tile([H, CW], BF16, tag=f"t1{c}", name=f"t1{t}_{c}_{rep}")
            nc.vector.scalar_tensor_tensor(
                t1_sb[:], phn, bhhn_sb[:, 0:1], r_sb[:],
                ALU.add, ALU.mult)
            t2_sb = run.tile([H, CW], F32, tag=f"t2{c}", name=f"t2{t}_{c}_{rep}")
            nc.vector.tensor_add(t2_sb[:], t1_sb[:], pxn)
            # off-backbone pieces
            oz_sb = run.tile([H, CW], BF16, tag=f"oz{c}",
                             name=f"oz{t}_{c}_{rep}")
            nc.vector.tensor_scalar(oz_sb[:], z_sb[:], -1.0, 1.0,
                                    ALU.mult, ALU.add)
            zh_sb = run.tile([H, CW], BF16, tag=f"zh{c}",
                             name=f"zh{t}_{c}_{rep}")
            nc.gpsimd.tensor_mul(zh_sb[:], z_sb[:], h_prev)
            g_state[c] = (t2_sb, oz_sb, zh_sb)

        def emit_gru_back(t, c):
            t2_sb, oz_sb, zh_sb = g_state[c]
            h_out = y[c][:, t * CW : (t + 1) * CW]
            n_sb = run.tile([H, CW], BF16, tag=f"n{c}", name=f"n{t}_{c}_{rep}")
            nc.scalar.activation(n_sb[:], t2_sb[:], AF.Tanh)
            nz_sb = run.tile([H, CW], BF16, tag=f"nz{c}", name=f"nz{t}_{c}_{rep}")
            nc.vector.tensor_mul(nz_sb[:], n_sb[:], oz_sb[:])
            nc.vector.tensor_add(h_out, nz_sb[:], zh_sb[:])

        pend_out = [None] * NCH  # deferred output projection: (t, hl tile)

        def flush_out(c):
            if pend_out[c] is None:
                return
            t_, hl_ = pend_out[c]
            pend_out[c] = None
            pout = P[c][0:V, 2 * CW : 3 * CW]
            mm(pout, outw_sb[:], hl_[:], start=True, stop=True)
            out_sb = run.tile([V, CW], F32, tag=f"out{c}",
                              name=f"out{t_}_{c}_{rep}", uniquify=True)
            nc.vector.tensor_copy(out_sb[:], pout)  # Pool can't read PSUM
            nc.sync.dma_start(d["logits"][t_][:, chs[c]], out_sb[:])

        l_state = [None] * NCH  # (cp, o_sb, hl_new) from lstm_front to back

        def emit_lstm_front(t, c):
            Pc = P[c]
            hl_prev = hl_t[c][t % 2]
            y_t = y[c][:, t * CW : (t + 1) * CW]
            # gate order [i, f, o, g]; psum regions i|f|o|g by bank
            pif = Pc[0:E, 0 : 2 * CW]
            po = Pc[0:E, 2 * CW : 3 * CW]
            pg = Pc[0:E, 3 * CW : 4 * CW]
            regions = [pif[:, 0:CW], pif[:, CW:], po, pg]
            # input projections (recurrence-independent) for i, f, g first,
            # then the deferred previous-step output projection (which reuses
            # the o bank), then the o input projection, then the recurrent
            # projections in backbone-criticality order i, f, g, o.
            for gi in (0, 1, 3):
                gs = slice(gi * E, (gi + 1) * E)
                mm(regions[gi], wihl_sb[:, gs], y_t, start=True, stop=False)
            flush_out(c)
            mm(regions[2], wihl_sb[:, 2 * E : 3 * E], y_t,
               start=True, stop=False)
            for gi in (0, 1, 3, 2):
                gs = slice(gi * E, (gi + 1) * E)
                mm(regions[gi], whhl_sb[:, gs], hl_prev[:],
                   start=False, stop=True)

            if_sb = run.tile([E, 2 * CW], BF16, tag=f"if{c}",
                             name=f"if{t}_{c}_{rep}")
            nc.scalar.activation(if_sb[:], pif, AF.Sigmoid)
            g_sb = run.tile([E, CW], BF16, tag=f"gg{c}", name=f"g{t}_{c}_{rep}")
            nc.scalar.activation(g_sb[:], pg, AF.Tanh)
            o_sb = run.tile([E, CW], BF16, tag=f"og{c}", name=f"o{t}_{c}_{rep}")
            nc.scalar.activation(o_sb[:], po, AF.Sigmoid)

            cp = run.tile([E, CW], F32, tag=f"cp{c}", name=f"cp{t}_{c}_{rep}")
            if t == 0:
                nc.vector.tensor_mul(cp[:], if_sb[:, 0:CW], g_sb[:])
            else:
                m1_sb = run.tile([E, CW], F32, tag=f"m1{c}",
                                 name=f"m1{t}_{c}_{rep}")
                nc.gpsimd.tensor_mul(m1_sb[:], if_sb[:, CW:], c_prev[c][:])
                m2_sb = run.tile([E, CW], F32, tag=f"m2{c}",
                                 name=f"m2{t}_{c}_{rep}")
                nc.vector.tensor_mul(m2_sb[:], if_sb[:, 0:CW], g_sb[:])
                nc.vector.tensor_add(cp[:], m1_sb[:], m2_sb[:])
            c_prev[c] = cp
            l_state[c] = (cp, o_sb, hl_t[c][(t + 1) % 2])

        def emit_lstm_back(t, c):
            cp, o_sb, hl_new = l_state[c]
            tc_sb = run.tile([E, CW], BF16, tag=f"tc{c}", name=f"tc{t}_{c}_{rep}")
            nc.scalar.activation(tc_sb[:], cp[:], AF.Tanh)
            nc.vector.tensor_mul(hl_new[0:E, :], o_sb[:], tc_sb[:])
            pend_out[c] = (t, hl_new)

        # Chain 1 is skewed one step behind chain 0 in both passes, and each
        # step is split front/back, so every instruction is data-ready by the
        # time the (in-order) engine queues reach it.
        for c in range(NCH):
            emit_h0(c)
        load_O(0)
        load_O(1)
        for t in range(T + 1):
            if t + 2 < T:
                load_O(t + 2)
            if t < T:
                emit_gru_front(t, 0)
            if t >= 1:
                emit_gru_front(t - 1, 1)
            if t < T:
                emit_gru_back(t, 0)
            if t >= 1:
                emit_gru_back(t - 1, 1)
        for t in range(T + 1):
            if t < T:
                emit_lstm_front(t, 0)
            if t >= 1:
                emit_lstm_front(t - 1, 1)
            if t < T:
                emit_lstm_back(t, 0)
            if t >= 1:
                emit_lstm_back(t - 1, 1)
        for c in range(NCH):
            flush_out(c)


def _host_prep(inputs):
    import ml_dtypes
    f32 = np.float32
    bf16 = ml_dtypes.bfloat16
    emb = np.asarray(inputs["emb"], f32)
    gru_wih = np.asarray(inputs["gru_wih"], f32)
    gru_whh = np.asarray(inputs["gru_whh"], f32)
    gru_bih = np.asarray(inputs["gru_bih"], f32)
    gru_bhh = np.asarray(inputs["gru_bhh"], f32)
    lstm_wih = np.asarray(inputs["lstm_wih"], f32)
    lstm_whh = np.asarray(inputs["lstm_whh"], f32)
    lstm_bih = np.asarray(inputs["lstm_bih"], f32)
    lstm_bhh = np.asarray(inputs["lstm_bhh"], f32)
    out_w = np.asarray(inputs["out_w"], f32)
    out_b = np.asarray(inputs["out_b"], f32)
    fc_z_w = np.asarray(inputs["fc_z_w"], f32)
    fc_z_b = np.asarray(inputs["fc_z_b"], f32)

    xg_tab = emb @ gru_wih.T + gru_bih
    xg_tab[:, 0:H] += gru_bhh[0:H]
    xg_tab[:, H : 2 * H] += gru_bhh[H : 2 * H]

    hl_init = np.zeros((E + 1, BC), f32)
    hl_init[E, :] = 1.0

    # Reorder LSTM gates [i, f, g, o] -> [i, f, o, g]
    perm = np.concatenate([np.arange(0, 2 * E), np.arange(3 * E, 4 * E),
                           np.arange(2 * E, 3 * E)])
    wih_l = lstm_wih[perm]
    whh_l = lstm_whh[perm]
    b_l = (lstm_bih + lstm_bhh)[perm]

    wih_lT = np.ascontiguousarray(wih_l.T)
    whh_laug = np.concatenate([whh_l.T, b_l[None, :]], axis=0)
    out_waug = np.concatenate([out_w.T, out_b[None, :]], axis=0)

    c = np.ascontiguousarray
    return {
        "hl_init": c(hl_init.astype(bf16)),
        "xg_tab": c(xg_tab.astype(bf16)),
        "bhh_n": c(gru_bhh[2 * H : 3 * H][:, None].astype(f32)),
        "whh_T": c(gru_whh.T.astype(bf16)),
        "fcw_T": c(fc_z_w.T.astype(f32)),
        "fc_b": c(fc_z_b[:, None].astype(f32)),
        "wih_lT": c(wih_lT.astype(bf16)),
        "whh_laug": c(whh_laug.astype(bf16)),
        "out_waug": c(out_waug.astype(bf16)),
    }


_NC_CACHE = {}


def _build(num_devices=N_CORES, reps=1):
    key = (num_devices, reps)
    if key in _NC_CACHE:
        return _NC_CACHE[key]
    nc = bacc.Bacc("TRN2", target_bir_lowering=False, debug=False,
                   num_devices=num_devices)
    d = {}
    for name, shape, dt_ in [
        ("zT", [L, BC], F32R), ("O", [T, V, BC], BF16),
        ("xg_tab", [V, 3 * H], BF16), ("bhh_n", [H, 1], F32),
        ("whh_T", [H, 3 * H], BF16),
        ("fcw_T", [L, H], F32R), ("fc_b", [H, 1], F32),
        ("wih_lT", [H, 4 * E], BF16), ("whh_laug", [E + 1, 4 * E], BF16),
        ("out_waug", [E + 1, V], BF16), ("hl_init", [E + 1, BC], BF16),
    ]:
        d[name] = nc.dram_tensor(name, shape, dt_, kind="ExternalInput").ap()
    d["logits"] = nc.dram_tensor("logits", [T, V, BC], F32,
                                 kind="ExternalOutput").ap()
    with tile.TileContext(nc) as tc:
        with ExitStack() as ctx:
            _emit(nc, tc, d, ctx, reps=reps)
    nc.compile()
    _NC_CACHE[key] = nc
    return nc


def build_in_maps(inputs):
    import ml_dtypes
    prep = _host_prep(inputs)
    z = np.asarray(inputs["z"], np.float32)
    x_in = np.asarray(inputs["x_in"])
    zT = np.ascontiguousarray(z.T)                       # (L, B)
    # one-hot [T, V, B] in bf16 (exact 0/1)
    O = (x_in[:, :, None] == np.arange(V)[None, None, :])
    O = np.ascontiguousarray(
        np.transpose(O, (1, 2, 0))).astype(ml_dtypes.bfloat16)  # (T, V, B)
    in_maps = []
    for ci in range(N_CORES):
        bs = slice(ci * BC, (ci + 1) * BC)
        m = dict(prep)
        m["zT"] = np.ascontiguousarray(zT[:, bs])
        m["O"] = np.ascontiguousarray(O[:, :, bs])
        in_maps.append(m)
    return in_maps


def assemble_output(results):
    outs = []
    for ci in range(N_CORES):
        lg = results[ci]["logits"]                       # (T, V, BC)
        outs.append(np.ascontiguousarray(np.transpose(lg, (2, 0, 1))))
    return np.concatenate(outs, axis=0).astype(np.float32)  # (B, T, V)


def kernel(**inputs) -> np.ndarray:
    nc = _build()
    in_maps = build_in_maps(inputs)
    res = run_bass_kernel_spmd(nc, in_maps, list(range(N_CORES)))
    return assemble_output(res.results)


# revision 17
# speedup vs baseline: 3.2210x; 3.2210x over previous
"""Trainium2 Bass kernel for nn_ARDecoderECD (GRU->LSTM AR decoder).

Strategy (pure data-parallel over 8 NeuronCores, batch-sharded):
  - layout: hidden dim on SBUF partitions, batch on the free dim
  - embedding + GRU input projection folded into a 23-row table applied via
    one-hot matmul (one-hot computed on host in bf16, streamed from DRAM)
  - 2 independent batch chains of 512 per core
  - TWO PASSES: all 26 GRU steps first (hidden states accumulate in a
    persistent SBUF bf16 buffer), then all 26 LSTM steps.  Each pass gets
    4 PSUM banks per chain with no cross-phase bank conflicts, so the two
    chains' serial recurrences overlap cleanly on the engines.
  - LSTM i,f,o gates land contiguously in PSUM -> single fused sigmoid
  - bf16 matmuls and bf16 SBUF elementwise (2x/4x DVE); LSTM c state in f32
  - elementwise spread over ACT / DVE / Pool to balance engine load
"""

import numpy as np
from contextlib import ExitStack

import concourse.bacc as bacc
import concourse.bass as bass
import concourse.tile as tile
from concourse import mybir
from concourse.bass_utils import run_bass_kernel_spmd

B, T = 8192, 26
V, E, H, L = 23, 100, 128, 64
N_CORES = 8
BC = B // N_CORES  # 1024 samples per core
F32 = mybir.dt.float32
F32R = mybir.dt.float32r
BF16 = mybir.dt.bfloat16
AF = mybir.ActivationFunctionType
ALU = mybir.AluOpType
NCH = 2
CW = BC // NCH  # 512


def _emit(nc, tc, d, ctx, reps=1):
    """Emit the per-core kernel. d maps names -> DRAM APs."""
    wp = ctx.enter_context(tc.tile_pool(name="weights", bufs=1))
    run = ctx.enter_context(tc.tile_pool(name="run", bufs=2))
    opool = ctx.enter_context(tc.tile_pool(name="opool", bufs=4))
    pp = ctx.enter_context(tc.tile_pool(name="psum", bufs=1, space="PSUM"))

    def mm(out, lhsT, rhs, start, stop):
        nc.tensor.matmul(out, lhsT, rhs, start=start, stop=stop)

    # ---- load weights ----
    def wload(name, shape, dt_):
        t = wp.tile(shape, dt_, name=name)
        nc.sync.dma_start(t[:], d[name][:])
        return t

    xg_sb = wload("xg_tab", [V, 3 * H], BF16)
    whh_sb = wload("whh_T", [H, 3 * H], BF16)
    fcw_sb = wload("fcw_T", [L, H], F32R)
    fcb_sb = wload("fc_b", [H, 1], F32)
    bhhn_sb = wload("bhh_n", [H, 1], F32)
    wihl_sb = wload("wih_lT", [H, 4 * E], BF16)
    whhl_sb = wload("whh_laug", [E + 1, 4 * E], BF16)
    outw_sb = wload("out_waug", [E + 1, V], BF16)
    zT_sb = wload("zT", [L, BC], F32R)

    chs = [slice(c * CW, (c + 1) * CW) for c in range(NCH)]

    # Persistent 4-bank PSUM tile per chain, manually sliced.
    P = [pp.tile([128, 4 * CW], F32, name=f"P{c}") for c in range(NCH)]
    # GRU hidden states for all T steps (per chain), bf16 in SBUF.
    y = [wp.tile([H, T * CW], BF16, name=f"y{c}") for c in range(NCH)]

    # LSTM state ping-pong tiles (per chain) with persistent ones-row (row E)
    hl_t = [[None, None] for _ in range(NCH)]
    for c in range(NCH):
        for idx in range(2):
            hlx = wp.tile([E + 1, CW], BF16, name=f"hl{c}_{idx}")
            nc.sync.dma_start(hlx[:], d["hl_init"][:, 0:CW])
            hl_t[c][idx] = hlx

    for rep in range(reps):
        h0 = [None] * NCH
        c_prev = [None] * NCH
        O_tiles = {}

        def load_O(t):
            Ot = opool.tile([V, BC], BF16, tag="O", name=f"O{t}_{rep}")
            nc.sync.dma_start(Ot[:], d["O"][t])
            O_tiles[t] = Ot

        def emit_h0(c):
            ph0 = P[c][:, 0:CW]
            mm(ph0, fcw_sb[:], zT_sb[:, chs[c]], start=True, stop=True)
            hc = run.tile([H, CW], BF16, tag=f"h{c}", name=f"h_init_{rep}_{c}")
            nc.scalar.activation(hc[:], ph0, AF.Tanh, bias=fcb_sb[:, 0:1])
            h0[c] = hc

        g_state = [None] * NCH  # (t2, oz, zh) handed from gru_front to gru_back

        def emit_gru_front(t, c):
            Ot = O_tiles[t]
            Pc = P[c]
            h_prev = h0[c][:] if t == 0 else y[c][:, (t - 1) * CW : t * CW]
            prz = Pc[:, 0 : 2 * CW]
            pxn = Pc[:, 2 * CW : 3 * CW]
            phn = Pc[:, 3 * CW : 4 * CW]
            mm(prz[:, 0:CW], xg_sb[:, 0:H], Ot[:, chs[c]],
               start=True, stop=False)
            mm(prz[:, CW:], xg_sb[:, H : 2 * H], Ot[:, chs[c]],
               start=True, stop=False)
            mm(pxn, xg_sb[:, 2 * H : 3 * H], Ot[:, chs[c]],
               start=True, stop=True)
            mm(prz[:, 0:CW], whh_sb[:, 0:H], h_prev, start=False, stop=True)
            mm(phn, whh_sb[:, 2 * H : 3 * H], h_prev, start=True, stop=True)
            mm(prz[:, CW:], whh_sb[:, H : 2 * H], h_prev,
               start=False, stop=True)

            # r-sigmoid alone gates the backbone; z is off-path
            r_sb = run.tile([H, CW], BF16, tag=f"r{c}", name=f"r{t}_{c}_{rep}")
            nc.scalar.activation(r_sb[:], prz[:, 0:CW], AF.Sigmoid)
            z_sb = run.tile([H, CW], BF16, tag=f"z{c}", name=f"z{t}_{c}_{rep}")
            nc.scalar.activation(z_sb[:], prz[:, CW:], AF.Sigmoid)
            # backbone: n-gate pre-activation
            t1_sb = run.tile([H, CW], BF16, tag=f"t1{c}", name=f"t1{t}_{c}_{rep}")
            nc.vector.scalar_tensor_tensor(
                t1_sb[:], phn, bhhn_sb[:, 0:1], r_sb[:],
                ALU.add, ALU.mult)
            t2_sb = run.tile([H, CW], F32, tag=f"t2{c}", name=f"t2{t}_{c}_{rep}")
            nc.vector.tensor_add(t2_sb[:], t1_sb[:], pxn)
            # off-backbone pieces
            oz_sb = run.tile([H, CW], BF16, tag=f"oz{c}",
                             name=f"oz{t}_{c}_{rep}")
            nc.vector.tensor_scalar(oz_sb[:], z_sb[:], -1.0, 1.0,
                                    ALU.mult, ALU.add)
            zh_sb = run.tile([H, CW], BF16, tag=f"zh{c}",
                             name=f"zh{t}_{c}_{rep}")
            nc.gpsimd.tensor_mul(zh_sb[:], z_sb[:], h_prev)
            g_state[c] = (t2_sb, oz_sb, zh_sb)

        def emit_gru_back(t, c):
            t2_sb, oz_sb, zh_sb = g_state[c]
            h_out = y[c][:, t * CW : (t + 1) * CW]
            n_sb = run.tile([H, CW], BF16, tag=f"n{c}", name=f"n{t}_{c}_{rep}")
            nc.scalar.activation(n_sb[:], t2_sb[:], AF.Tanh)
            nz_sb = run.tile([H, CW], BF16, tag=f"nz{c}", name=f"nz{t}_{c}_{rep}")
            nc.vector.tensor_mul(nz_sb[:], n_sb[:], oz_sb[:])
            nc.vector.tensor_add(h_out, nz_sb[:], zh_sb[:])

        pend_out = [None] * NCH  # deferred output projection: (t, hl tile)

        def flush_out(c):
            if pend_out[c] is None:
                return
            t_, hl_ = pend_out[c]
            pend_out[c] = None
            pout = P[c][0:V, 2 * CW : 3 * CW]
            mm(pout, outw_sb[:], hl_[:], start=True, stop=True)
            out_sb = run.tile([V, CW], F32, tag=f"out{c}",
                              name=f"out{t_}_{c}_{rep}", uniquify=True)
            nc.vector.tensor_copy(out_sb[:], pout)  # Pool can't read PSUM
            nc.sync.dma_start(d["logits"][t_][:, chs[c]], out_sb[:])

        l_state = [None] * NCH  # (cp, o_sb, hl_new) from lstm_front to back

        def emit_lstm_front(t, c):
            Pc = P[c]
            hl_prev = hl_t[c][t % 2]
            y_t = y[c][:, t * CW : (t + 1) * CW]
            # gate order [i, f, o, g]; psum regions i|f|o|g by bank
            pif = Pc[0:E, 0 : 2 * CW]
            po = Pc[0:E, 2 * CW : 3 * CW]
            pg = Pc[0:E, 3 * CW : 4 * CW]
            regions = [pif[:, 0:CW], pif[:, CW:], po, pg]
            # input projections (recurrence-independent) for i, f, g first,
            # then the deferred previous-step output projection (which reuses
            # the o bank), then the o input projection, then the recurrent
            # projections in backbone-criticality order i, f, g, o.
            for gi in (0, 1, 3):
                gs = slice(gi * E, (gi + 1) * E)
                mm(regions[gi], wihl_sb[:, gs], y_t, start=True, stop=False)
            flush_out(c)
            mm(regions[2], wihl_sb[:, 2 * E : 3 * E], y_t,
               start=True, stop=False)
            for gi in (0, 1, 3, 2):
                gs = slice(gi * E, (gi + 1) * E)
                mm(regions[gi], whhl_sb[:, gs], hl_prev[:],
                   start=False, stop=True)

            if_sb = run.tile([E, 2 * CW], BF16, tag=f"if{c}",
                             name=f"if{t}_{c}_{rep}")
            nc.scalar.activation(if_sb[:], pif, AF.Sigmoid)
            g_sb = run.tile([E, CW], BF16, tag=f"gg{c}", name=f"g{t}_{c}_{rep}")
            nc.scalar.activation(g_sb[:], pg, AF.Tanh)
            o_sb = run.tile([E, CW], BF16, tag=f"og{c}", name=f"o{t}_{c}_{rep}")
            nc.scalar.activation(o_sb[:], po, AF.Sigmoid)

            cp = run.tile([E, CW], F32, tag=f"cp{c}", name=f"cp{t}_{c}_{rep}")
            if t == 0:
                nc.vector.tensor_mul(cp[:], if_sb[:, 0:CW], g_sb[:])
            else:
                m1_sb = run.tile([E, CW], F32, tag=f"m1{c}",
                                 name=f"m1{t}_{c}_{rep}")
                nc.gpsimd.tensor_mul(m1_sb[:], if_sb[:, CW:], c_prev[c][:])
                m2_sb = run.tile([E, CW], F32, tag=f"m2{c}",
                                 name=f"m2{t}_{c}_{rep}")
                nc.vector.tensor_mul(m2_sb[:], if_sb[:, 0:CW], g_sb[:])
                nc.vector.tensor_add(cp[:], m1_sb[:], m2_sb[:])
            c_prev[c] = cp
            l_state[c] = (cp, o_sb, hl_t[c][(t + 1) % 2])

        def emit_lstm_back(t, c):
            cp, o_sb, hl_new = l_state[c]
            tc_sb = run.tile([E, CW], BF16, tag=f"tc{c}", name=f"tc{t}_{c}_{rep}")
            nc.scalar.activation(tc_sb[:], cp[:], AF.Tanh)
            nc.vector.tensor_mul(hl_new[0:E, :], o_sb[:], tc_sb[:])
            pend_out[c] = (t, hl_new)

        # Chain 1 is skewed one step behind chain 0 in both passes, and each
        # step is split front/back, so every instruction is data-ready by the
        # time the (in-order) engine queues reach it.
        for c in range(NCH):
            emit_h0(c)
        load_O(0)
        load_O(1)
        for t in range(T + 1):
            if t + 2 < T:
                load_O(t + 2)
            if t < T:
                emit_gru_front(t, 0)
            if t >= 1:
                emit_gru_front(t - 1, 1)
            if t < T:
                emit_gru_back(t, 0)
            if t >= 1:
                emit_gru_back(t - 1, 1)
        for t in range(T + 1):
            if t < T:
                emit_lstm_front(t, 0)
            if t >= 1:
                emit_lstm_front(t - 1, 1)
            if t < T:
                emit_lstm_back(t, 0)
            if t >= 1:
                emit_lstm_back(t - 1, 1)
        for c in range(NCH):
            flush_out(c)


def _host_prep(inputs):
    import ml_dtypes
    f32 = np.float32
    bf16 = ml_dtypes.bfloat16
    emb = np.asarray(inputs["emb"], f32)
    gru_wih = np.asarray(inputs["gru_wih"], f32)
    gru_whh = np.asarray(inputs["gru_whh"], f32)
    gru_bih = np.asarray(inputs["gru_bih"], f32)
    gru_bhh = np.asarray(inputs["gru_bhh"], f32)
    lstm_wih = np.asarray(inputs["lstm_wih"], f32)
    lstm_whh = np.asarray(inputs["lstm_whh"], f32)
    lstm_bih = np.asarray(inputs["lstm_bih"], f32)
    lstm_bhh = np.asarray(inputs["lstm_bhh"], f32)
    out_w = np.asarray(inputs["out_w"], f32)
    out_b = np.asarray(inputs["out_b"], f32)
    fc_z_w = np.asarray(inputs["fc_z_w"], f32)
    fc_z_b = np.asarray(inputs["fc_z_b"], f32)

    xg_tab = emb @ gru_wih.T + gru_bih
    xg_tab[:, 0:H] += gru_bhh[0:H]
    xg_tab[:, H : 2 * H] += gru_bhh[H : 2 * H]

    hl_init = np.zeros((E + 1, BC), f32)
    hl_init[E, :] = 1.0

    # Reorder LSTM gates [i, f, g, o] -> [i, f, o, g]
    perm = np.concatenate([np.arange(0, 2 * E), np.arange(3 * E, 4 * E),
                           np.arange(2 * E, 3 * E)])
    wih_l = lstm_wih[perm]
    whh_l = lstm_whh[perm]
    b_l = (lstm_bih + lstm_bhh)[perm]

    wih_lT = np.ascontiguousarray(wih_l.T)
    whh_laug = np.concatenate([whh_l.T, b_l[None, :]], axis=0)
    out_waug = np.concatenate([out_w.T, out_b[None, :]], axis=0)

    c = np.ascontiguousarray
    return {
        "hl_init": c(hl_init.astype(bf16)),
        "xg_tab": c(xg_tab.astype(bf16)),
        "bhh_n": c(gru_bhh[2 * H : 3 * H][:, None].astype(f32)),
        "whh_T": c(gru_whh.T.astype(bf16)),
        "fcw_T": c(fc_z_w.T.astype(f32)),
        "fc_b": c(fc_z_b[:, None].astype(f32)),
        "wih_lT": c(wih_lT.astype(bf16)),
        "whh_laug": c(whh_laug.astype(bf16)),
        "out_waug": c(out_waug.astype(bf16)),
    }


_NC_CACHE = {}


def _build(num_devices=N_CORES, reps=1):
    key = (num_devices, reps)
    if key in _NC_CACHE:
        return _NC_CACHE[key]
    nc = bacc.Bacc("TRN2", target_bir_lowering=False, debug=False,
                   num_devices=num_devices)
    d = {}
    for name, shape, dt_ in [
        ("zT", [L, BC], F32R), ("O", [T, V, BC], BF16),
        ("xg_tab", [V, 3 * H], BF16), ("bhh_n", [H, 1], F32),
        ("whh_T", [H, 3 * H], BF16),
        ("fcw_T", [L, H], F32R), ("fc_b", [H, 1], F32),
        ("wih_lT", [H, 4 * E], BF16), ("whh_laug", [E + 1, 4 * E], BF16),
        ("out_waug", [E + 1, V], BF16), ("hl_init", [E + 1, BC], BF16),
    ]:
        d[name] = nc.dram_tensor(name, shape, dt_, kind="ExternalInput").ap()
    d["logits"] = nc.dram_tensor("logits", [T, V, BC], F32,
                                 kind="ExternalOutput").ap()
    with tile.TileContext(nc) as tc:
        with ExitStack() as ctx:
            _emit(nc, tc, d, ctx, reps=reps)
    nc.compile()
    _NC_CACHE[key] = nc
    return nc


def build_in_maps(inputs):
    import ml_dtypes
    prep = _host_prep(inputs)
    z = np.asarray(inputs["z"], np.float32)
    x_in = np.asarray(inputs["x_in"])
    zT = np.ascontiguousarray(z.T)                       # (L, B)
    # one-hot [T, V, B] in bf16 (exact 0/1)
    O = (x_in[:, :, None] == np.arange(V)[None, None, :])
    O = np.ascontiguousarray(
        np.transpose(O, (1, 2, 0))).astype(ml_dtypes.bfloat16)  # (T, V, B)
    in_maps = []
    for ci in range(N_CORES):
        bs = slice(ci * BC, (ci + 1) * BC)
        m = dict(prep)
        m["zT"] = np.ascontiguousarray(zT[:, bs])
        m["O"] = np.ascontiguousarray(O[:, :, bs])
        in_maps.append(m)
    return in_maps


def assemble_output(results):
    outs = []
    for ci in range(N_CORES):
        lg = results[ci]["logits"]                       # (T, V, BC)
        outs.append(np.ascontiguousarray(np.transpose(lg, (2, 0, 1))))
    return np.concatenate(outs, axis=0).astype(np.float32)  # (B, T, V)


def kernel(**inputs) -> np.ndarray:
    nc = _build()
    in_maps = build_in_maps(inputs)
    res = run_bass_kernel_spmd(nc, in_maps, list(range(N_CORES)))
    return assemble_output(res.results)


# revision 18
# speedup vs baseline: 3.3845x; 1.0508x over previous
"""Trainium2 Bass kernel for nn_ARDecoderECD (GRU->LSTM AR decoder).

Strategy (pure data-parallel over 8 NeuronCores, batch-sharded):
  - layout: hidden dim on SBUF partitions, batch on the free dim
  - embedding + GRU input projection folded into a 23-row table applied via
    one-hot matmul (one-hot computed on host in bf16, streamed from DRAM)
  - 2 independent batch chains of 512 per core
  - TWO PASSES: all 26 GRU steps first (hidden states accumulate in a
    persistent SBUF bf16 buffer), then all 26 LSTM steps.  Each pass gets
    4 PSUM banks per chain with no cross-phase bank conflicts, so the two
    chains' serial recurrences overlap cleanly on the engines.
  - LSTM i,f,o gates land contiguously in PSUM -> single fused sigmoid
  - bf16 matmuls and bf16 SBUF elementwise (2x/4x DVE); LSTM c state in f32
  - elementwise spread over ACT / DVE / Pool to balance engine load
"""

import numpy as np
from contextlib import ExitStack

import concourse.bacc as bacc
import concourse.bass as bass
import concourse.tile as tile
from concourse import mybir
from concourse.bass_utils import run_bass_kernel_spmd

B, T = 8192, 26
V, E, H, L = 23, 100, 128, 64
N_CORES = 8
BC = B // N_CORES  # 1024 samples per core
F32 = mybir.dt.float32
F32R = mybir.dt.float32r
BF16 = mybir.dt.bfloat16
AF = mybir.ActivationFunctionType
ALU = mybir.AluOpType
NCH = 2
CW = BC // NCH  # 512


def _emit(nc, tc, d, ctx, reps=1):
    """Emit the per-core kernel. d maps names -> DRAM APs."""
    wp = ctx.enter_context(tc.tile_pool(name="weights", bufs=1))
    run = ctx.enter_context(tc.tile_pool(name="run", bufs=2))
    opool = ctx.enter_context(tc.tile_pool(name="opool", bufs=4))
    pp = ctx.enter_context(tc.tile_pool(name="psum", bufs=1, space="PSUM"))

    def mm(out, lhsT, rhs, start, stop):
        nc.tensor.matmul(out, lhsT, rhs, start=start, stop=stop)

    # ---- load weights ----
    def wload(name, shape, dt_):
        t = wp.tile(shape, dt_, name=name)
        nc.sync.dma_start(t[:], d[name][:])
        return t

    xg_sb = wload("xg_tab", [V, 3 * H], BF16)
    whh_sb = wload("whh_T", [H, 3 * H], BF16)
    fcw_sb = wload("fcw_T", [L, H], F32R)
    fcb_sb = wload("fc_b", [H, 1], F32)
    bhhn_sb = wload("bhh_n", [H, 1], F32)
    wihl_sb = wload("wih_lT", [H, 4 * E], BF16)
    whhl_sb = wload("whh_laug", [E + 1, 4 * E], BF16)
    outw_sb = wload("out_waug", [E + 1, V], BF16)
    zT_sb = wload("zT", [L, BC], F32R)

    chs = [slice(c * CW, (c + 1) * CW) for c in range(NCH)]

    # Persistent 4-bank PSUM tile per chain, manually sliced.
    P = [pp.tile([128, 4 * CW], F32, name=f"P{c}") for c in range(NCH)]
    # GRU hidden states for all T steps (per chain), bf16 in SBUF.
    y = [wp.tile([H, T * CW], BF16, name=f"y{c}") for c in range(NCH)]

    # LSTM state ping-pong tiles (per chain) with persistent ones-row (row E)
    hl_t = [[None, None] for _ in range(NCH)]
    for c in range(NCH):
        for idx in range(2):
            hlx = wp.tile([E + 1, CW], BF16, name=f"hl{c}_{idx}")
            nc.sync.dma_start(hlx[:], d["hl_init"][:, 0:CW])
            hl_t[c][idx] = hlx

    for rep in range(reps):
        h0 = [None] * NCH
        c_prev = [None] * NCH
        O_tiles = {}

        def load_O(t):
            Ot = opool.tile([V, BC], BF16, tag="O", name=f"O{t}_{rep}")
            nc.sync.dma_start(Ot[:], d["O"][t])
            O_tiles[t] = Ot

        def emit_h0(c):
            ph0 = P[c][:, 0:CW]
            mm(ph0, fcw_sb[:], zT_sb[:, chs[c]], start=True, stop=True)
            hc = run.tile([H, CW], BF16, tag=f"h{c}", name=f"h_init_{rep}_{c}")
            nc.scalar.activation(hc[:], ph0, AF.Tanh, bias=fcb_sb[:, 0:1])
            h0[c] = hc

        g_state = [None] * NCH  # (t2, oz, zh) handed from gru_front to gru_back

        def emit_gru_front(t, c):
            Ot = O_tiles[t]
            Pc = P[c]
            h_prev = h0[c][:] if t == 0 else y[c][:, (t - 1) * CW : t * CW]
            prz = Pc[:, 0 : 2 * CW]
            pxn = Pc[:, 2 * CW : 3 * CW]
            phn = Pc[:, 3 * CW : 4 * CW]
            mm(prz[:, 0:CW], xg_sb[:, 0:H], Ot[:, chs[c]],
               start=True, stop=False)
            mm(prz[:, CW:], xg_sb[:, H : 2 * H], Ot[:, chs[c]],
               start=True, stop=False)
            mm(pxn, xg_sb[:, 2 * H : 3 * H], Ot[:, chs[c]],
               start=True, stop=True)
            mm(prz[:, 0:CW], whh_sb[:, 0:H], h_prev, start=False, stop=True)
            mm(phn, whh_sb[:, 2 * H : 3 * H], h_prev, start=True, stop=True)
            mm(prz[:, CW:], whh_sb[:, H : 2 * H], h_prev,
               start=False, stop=True)

            # r-sigmoid alone gates the backbone; z is off-path
            r_sb = run.tile([H, CW], BF16, tag=f"r{c}", name=f"r{t}_{c}_{rep}")
            nc.scalar.activation(r_sb[:], prz[:, 0:CW], AF.Sigmoid)
            z_sb = run.tile([H, CW], BF16, tag=f"z{c}", name=f"z{t}_{c}_{rep}")
            nc.scalar.activation(z_sb[:], prz[:, CW:], AF.Sigmoid)
            # backbone: n-gate pre-activation
            t1_sb = run.tile([H, CW], BF16, tag=f"t1{c}", name=f"t1{t}_{c}_{rep}")
            nc.vector.scalar_tensor_tensor(
                t1_sb[:], phn, bhhn_sb[:, 0:1], r_sb[:],
                ALU.add, ALU.mult)
            t2_sb = run.tile([H, CW], F32, tag=f"t2{c}", name=f"t2{t}_{c}_{rep}")
            nc.vector.tensor_add(t2_sb[:], t1_sb[:], pxn)
            # off-backbone pieces
            oz_sb = run.tile([H, CW], BF16, tag=f"oz{c}",
                             name=f"oz{t}_{c}_{rep}")
            nc.vector.tensor_scalar(oz_sb[:], z_sb[:], -1.0, 1.0,
                                    ALU.mult, ALU.add)
            zh_sb = run.tile([H, CW], BF16, tag=f"zh{c}",
                             name=f"zh{t}_{c}_{rep}")
            nc.gpsimd.tensor_mul(zh_sb[:], z_sb[:], h_prev)
            g_state[c] = (t2_sb, oz_sb, zh_sb)

        def emit_gru_back(t, c):
            t2_sb, oz_sb, zh_sb = g_state[c]
            h_out = y[c][:, t * CW : (t + 1) * CW]
            n_sb = run.tile([H, CW], BF16, tag=f"n{c}", name=f"n{t}_{c}_{rep}")
            nc.scalar.activation(n_sb[:], t2_sb[:], AF.Tanh)
            nz_sb = run.tile([H, CW], BF16, tag=f"nz{c}", name=f"nz{t}_{c}_{rep}")
            nc.vector.tensor_mul(nz_sb[:], n_sb[:], oz_sb[:])
            nc.vector.tensor_add(h_out, nz_sb[:], zh_sb[:])

        pend_out = [None] * NCH  # deferred output projection: (t, hl tile)

        def flush_out(c):
            if pend_out[c] is None:
                return
            t_, hl_ = pend_out[c]
            pend_out[c] = None
            pout = P[c][0:V, 2 * CW : 3 * CW]
            mm(pout, outw_sb[:], hl_[:], start=True, stop=True)
            out_sb = run.tile([V, CW], F32, tag=f"out{c}",
                              name=f"out{t_}_{c}_{rep}", uniquify=True)
            nc.vector.tensor_copy(out_sb[:], pout)  # Pool can't read PSUM
            nc.sync.dma_start(d["logits"][t_][:, chs[c]], out_sb[:])

        l_state = [None] * NCH  # (cp, o_sb, hl_new) from lstm_front to back

        def emit_lstm_front(t, c):
            Pc = P[c]
            hl_prev = hl_t[c][t % 2]
            y_t = y[c][:, t * CW : (t + 1) * CW]
            # gate order [i, f, o, g]; psum regions i|f|o|g by bank
            pif = Pc[0:E, 0 : 2 * CW]
            po = Pc[0:E, 2 * CW : 3 * CW]
            pg = Pc[0:E, 3 * CW : 4 * CW]
            regions = [pif[:, 0:CW], pif[:, CW:], po, pg]
            # input projections (recurrence-independent) for i, f, g first,
            # then the deferred previous-step output projection (which reuses
            # the o bank), then the o input projection, then the recurrent
            # projections in backbone-criticality order i, f, g, o.
            for gi in (0, 1, 3):
                gs = slice(gi * E, (gi + 1) * E)
                mm(regions[gi], wihl_sb[:, gs], y_t, start=True, stop=False)
            flush_out(c)
            mm(regions[2], wihl_sb[:, 2 * E : 3 * E], y_t,
               start=True, stop=False)
            for gi in (0, 1, 3, 2):
                gs = slice(gi * E, (gi + 1) * E)
                mm(regions[gi], whhl_sb[:, gs], hl_prev[:],
                   start=False, stop=True)

            if_sb = run.tile([E, 2 * CW], BF16, tag=f"if{c}",
                             name=f"if{t}_{c}_{rep}")
            nc.scalar.activation(if_sb[:], pif, AF.Sigmoid)
            g_sb = run.tile([E, CW], BF16, tag=f"gg{c}", name=f"g{t}_{c}_{rep}")
            nc.scalar.activation(g_sb[:], pg, AF.Tanh)
            o_sb = run.tile([E, CW], BF16, tag=f"og{c}", name=f"o{t}_{c}_{rep}")
            nc.scalar.activation(o_sb[:], po, AF.Sigmoid)

            # c-state in bf16: all-SBUF 2-byte ops get 2x/4x DVE throughput,
            # shortening the serial c-chain (precision margin allows it)
            cp = run.tile([E, CW], BF16, tag=f"cp{c}", name=f"cp{t}_{c}_{rep}")
            if t == 0:
                nc.vector.tensor_mul(cp[:], if_sb[:, 0:CW], g_sb[:])
            else:
                m1_sb = run.tile([E, CW], BF16, tag=f"m1{c}",
                                 name=f"m1{t}_{c}_{rep}")
                nc.vector.tensor_mul(m1_sb[:], if_sb[:, CW:], c_prev[c][:])
                m2_sb = run.tile([E, CW], BF16, tag=f"m2{c}",
                                 name=f"m2{t}_{c}_{rep}")
                nc.vector.tensor_mul(m2_sb[:], if_sb[:, 0:CW], g_sb[:])
                nc.vector.tensor_add(cp[:], m1_sb[:], m2_sb[:])
            c_prev[c] = cp
            l_state[c] = (cp, o_sb, hl_t[c][(t + 1) % 2])

        def emit_lstm_back(t, c):
            cp, o_sb, hl_new = l_state[c]
            tc_sb = run.tile([E, CW], BF16, tag=f"tc{c}", name=f"tc{t}_{c}_{rep}")
            nc.scalar.activation(tc_sb[:], cp[:], AF.Tanh)
            nc.vector.tensor_mul(hl_new[0:E, :], o_sb[:], tc_sb[:])
            pend_out[c] = (t, hl_new)

        # Chain 1 is skewed one step behind chain 0 in both passes, and each
        # step is split front/back, so every instruction is data-ready by the
        # time the (in-order) engine queues reach it.
        for c in range(NCH):
            emit_h0(c)
        load_O(0)
        load_O(1)
        for t in range(T + 1):
            if t + 2 < T:
                load_O(t + 2)
            if t < T:
                emit_gru_front(t, 0)
            if t >= 1:
                emit_gru_front(t - 1, 1)
            if t < T:
                emit_gru_back(t, 0)
            if t >= 1:
                emit_gru_back(t - 1, 1)
        for t in range(T + 1):
            if t < T:
                emit_lstm_front(t, 0)
            if t >= 1:
                emit_lstm_front(t - 1, 1)
            if t < T:
                emit_lstm_back(t, 0)
            if t >= 1:
                emit_lstm_back(t - 1, 1)
        for c in range(NCH):
            flush_out(c)


def _host_prep(inputs):
    import ml_dtypes
    f32 = np.float32
    bf16 = ml_dtypes.bfloat16
    emb = np.asarray(inputs["emb"], f32)
    gru_wih = np.asarray(inputs["gru_wih"], f32)
    gru_whh = np.asarray(inputs["gru_whh"], f32)
    gru_bih = np.asarray(inputs["gru_bih"], f32)
    gru_bhh = np.asarray(inputs["gru_bhh"], f32)
    lstm_wih = np.asarray(inputs["lstm_wih"], f32)
    lstm_whh = np.asarray(inputs["lstm_whh"], f32)
    lstm_bih = np.asarray(inputs["lstm_bih"], f32)
    lstm_bhh = np.asarray(inputs["lstm_bhh"], f32)
    out_w = np.asarray(inputs["out_w"], f32)
    out_b = np.asarray(inputs["out_b"], f32)
    fc_z_w = np.asarray(inputs["fc_z_w"], f32)
    fc_z_b = np.asarray(inputs["fc_z_b"], f32)

    xg_tab = emb @ gru_wih.T + gru_bih
    xg_tab[:, 0:H] += gru_bhh[0:H]
    xg_tab[:, H : 2 * H] += gru_bhh[H : 2 * H]

    hl_init = np.zeros((E + 1, BC), f32)
    hl_init[E, :] = 1.0

    # Reorder LSTM gates [i, f, g, o] -> [i, f, o, g]
    perm = np.concatenate([np.arange(0, 2 * E), np.arange(3 * E, 4 * E),
                           np.arange(2 * E, 3 * E)])
    wih_l = lstm_wih[perm]
    whh_l = lstm_whh[perm]
    b_l = (lstm_bih + lstm_bhh)[perm]

    wih_lT = np.ascontiguousarray(wih_l.T)
    whh_laug = np.concatenate([whh_l.T, b_l[None, :]], axis=0)
    out_waug = np.concatenate([out_w.T, out_b[None, :]], axis=0)

    c = np.ascontiguousarray
    return {
        "hl_init": c(hl_init.astype(bf16)),
        "xg_tab": c(xg_tab.astype(bf16)),
        "bhh_n": c(gru_bhh[2 * H : 3 * H][:, None].astype(f32)),
        "whh_T": c(gru_whh.T.astype(bf16)),
        "fcw_T": c(fc_z_w.T.astype(f32)),
        "fc_b": c(fc_z_b[:, None].astype(f32)),
        "wih_lT": c(wih_lT.astype(bf16)),
        "whh_laug": c(whh_laug.astype(bf16)),
        "out_waug": c(out_waug.astype(bf16)),
    }


_NC_CACHE = {}


def _build(num_devices=N_CORES, reps=1):
    key = (num_devices, reps)
    if key in _NC_CACHE:
        return _NC_CACHE[key]
    nc = bacc.Bacc("TRN2", target_bir_lowering=False, debug=False,
                   num_devices=num_devices)
    d = {}
    for name, shape, dt_ in [
        ("zT", [L, BC], F32R), ("O", [T, V, BC], BF16),
        ("xg_tab", [V, 3 * H], BF16), ("bhh_n", [H, 1], F32),
        ("whh_T", [H, 3 * H], BF16),
        ("fcw_T", [L, H], F32R), ("fc_b", [H, 1], F32),
        ("wih_lT", [H, 4 * E], BF16), ("whh_laug", [E + 1, 4 * E], BF16),
        ("out_waug", [E + 1, V], BF16), ("hl_init", [E + 1, BC], BF16),
    ]:
        d[name] = nc.dram_tensor(name, shape, dt_, kind="ExternalInput").ap()
    d["logits"] = nc.dram_tensor("logits", [T, V, BC], F32,
                                 kind="ExternalOutput").ap()
    with tile.TileContext(nc) as tc:
        with ExitStack() as ctx:
            _emit(nc, tc, d, ctx, reps=reps)
    nc.compile()
    _NC_CACHE[key] = nc
    return nc


def build_in_maps(inputs):
    import ml_dtypes
    prep = _host_prep(inputs)
    z = np.asarray(inputs["z"], np.float32)
    x_in = np.asarray(inputs["x_in"])
    zT = np.ascontiguousarray(z.T)                       # (L, B)
    # one-hot [T, V, B] in bf16 (exact 0/1)
    O = (x_in[:, :, None] == np.arange(V)[None, None, :])
    O = np.ascontiguousarray(
        np.transpose(O, (1, 2, 0))).astype(ml_dtypes.bfloat16)  # (T, V, B)
    in_maps = []
    for ci in range(N_CORES):
        bs = slice(ci * BC, (ci + 1) * BC)
        m = dict(prep)
        m["zT"] = np.ascontiguousarray(zT[:, bs])
        m["O"] = np.ascontiguousarray(O[:, :, bs])
        in_maps.append(m)
    return in_maps


def assemble_output(results):
    outs = []
    for ci in range(N_CORES):
        lg = results[ci]["logits"]                       # (T, V, BC)
        outs.append(np.ascontiguousarray(np.transpose(lg, (2, 0, 1))))
    return np.concatenate(outs, axis=0).astype(np.float32)  # (B, T, V)


def kernel(**inputs) -> np.ndarray:
    nc = _build()
    in_maps = build_in_maps(inputs)
    res = run_bass_kernel_spmd(nc, in_maps, list(range(N_CORES)))
    return assemble_output(res.results)
